# revision 5
# baseline (speedup 1.0000x reference)
"""BiRNN (Bowman SNLI) Trainium2 kernel.

Full inputs -> full logits [256, 3].

Sharding: 8 cores = 2 batch halves x 4 LSTM runs (p_fw, p_bw, h_fw, h_bw).
Each core runs one masked-LSTM direction over its 128-row batch half,
entirely on-chip (input projection fused into the per-step PSUM gate
accumulation), then the four final cell states of each half are
AllGathered and every core computes the 4-layer feed-forward head for
its half; the host reads logits from cores 0 and 4.

All matmuls run as float32r (tf32-like). The time axis is reversed on
the host for backward cores so all cores execute an identical program.
Sequence-length masking folds into the i/f gate pre-activation biases.
"""
import os
import sys
from contextlib import ExitStack

sys.path.insert(0, "/opt/trn_rl_repo")

import numpy as np

import concourse.bass as bass
import concourse.mybir as mybir
import concourse.tile as tile
from concourse import bacc
from concourse import bass_utils

f32 = mybir.dt.float32
f32r = mybir.dt.float32r
AF = mybir.ActivationFunctionType

B = 256
T = int(os.environ.get("KBENCH_T", "256"))
D = 300
H = 512
FFD = 1024
FORGET_BIAS = 1.0
BIG = 30.0
NB = 128          # batch rows per core
G4 = 4 * H        # 2048 gate width
NKX = 3           # ceil(301/128) input-proj K chunks
KX_LAST = 45      # rows used in last x chunk (44 x rows + ones row)
NKH = 4           # H/128 recurrent K chunks


def build(with_ff=True):
    nc = bacc.Bacc("TRN2", num_devices=8)

    xT = nc.dram_tensor("xT", [T, NKX, 128, 128], f32r, kind="ExternalInput")
    wx = nc.dram_tensor("wx", [NKX, 128, G4], f32r, kind="ExternalInput")
    wh = nc.dram_tensor("wh", [NKH, 128, G4], f32r, kind="ExternalInput")
    addi = nc.dram_tensor("addi", [128, T], f32, kind="ExternalInput")
    addf = nc.dram_tensor("addf", [128, T], f32, kind="ExternalInput")
    identd = nc.dram_tensor("identd", [128, 128], f32r, kind="ExternalInput")
    onesd = nc.dram_tensor("onesd", [1, 128], f32r, kind="ExternalInput")
    w1 = nc.dram_tensor("w1", [16, 128, FFD], f32r, kind="ExternalInput")
    w2 = nc.dram_tensor("w2", [8, 128, FFD], f32r, kind="ExternalInput")
    w3 = nc.dram_tensor("w3", [8, 128, FFD], f32r, kind="ExternalInput")
    w4 = nc.dram_tensor("w4", [8, 128, 4], f32r, kind="ExternalInput")
    bff = nc.dram_tensor("bff", [1, 3 * FFD + 4], f32r, kind="ExternalInput")

    cout = nc.dram_tensor("cout", [128, H], f32, kind="ExternalOutput")
    logits = nc.dram_tensor("logits", [128, 4], f32, kind="ExternalOutput")

    with tile.TileContext(nc) as tc, ExitStack() as es:
        kpool = es.enter_context(tc.tile_pool(name="keep", bufs=1))
        dpool = es.enter_context(tc.tile_pool(name="ffdram", bufs=1, space="DRAM"))
        ident = kpool.tile([128, 128], f32r)
        ones1 = kpool.tile([1, 128], f32r)
        nc.sync.dma_start(ident[:], identd[:])
        nc.sync.dma_start(ones1[:], onesd[:])

        lstm_es = ExitStack()
        cpool = lstm_es.enter_context(tc.tile_pool(name="const", bufs=1))
        spool = lstm_es.enter_context(tc.tile_pool(name="state", bufs=2))
        xpool = lstm_es.enter_context(tc.tile_pool(name="xin", bufs=4))
        apool = lstm_es.enter_context(tc.tile_pool(name="gact", bufs=2))
        tpool = lstm_es.enter_context(tc.tile_pool(name="tmp", bufs=2))
        gpool = lstm_es.enter_context(tc.tile_pool(name="gpsum", bufs=6, space="PSUM"))
        ppool = lstm_es.enter_context(tc.tile_pool(name="tpsum", bufs=2, space="PSUM"))

        wxt = cpool.tile([128, NKX * G4], f32r)
        wht = cpool.tile([128, NKH * G4], f32r)
        ait = cpool.tile([128, T], f32)
        aft = cpool.tile([128, T], f32)
        for c in range(NKX):
            nc.sync.dma_start(wxt[:, c * G4:(c + 1) * G4], wx[c])
        for k in range(NKH):
            nc.sync.dma_start(wht[:, k * G4:(k + 1) * G4], wh[k])
        nc.sync.dma_start(ait[:], addi[:])
        nc.sync.dma_start(aft[:], addf[:])

        def wxc(c, g):
            return wxt[:KX_LAST if c == NKX - 1 else 128,
                       c * G4 + g * H:c * G4 + g * H + H]

        def whc(k, g):
            return wht[:, k * G4 + g * H:k * G4 + g * H + H]

        # ---------------- LSTM over time ----------------
        c_t = None
        hT_t = None            # [128, 512] f32r: 4 chunks of h^T

        def emit_x(t):
            """Load x_t^T and start gate accumulation for step t."""
            xt = xpool.tile([128, NKX * 128], f32r, tag="xt")
            for c in range(NKX):
                nc.sync.dma_start(xt[:, c * 128:(c + 1) * 128], xT[t, c])
            ng = 4 if t < T - 1 else 3   # final step: skip o gate
            gs = []
            for g in range(ng):
                pg = gpool.tile([128, H], f32, tag="gate")
                for c in range(NKX):
                    kk = KX_LAST if c == NKX - 1 else 128
                    nc.tensor.matmul(
                        pg[:], xt[:kk, c * 128:(c + 1) * 128], wxc(c, g),
                        start=(c == 0), stop=(c == NKX - 1 and t == 0),
                    )
                gs.append(pg)
            return gs

        gates = emit_x(0)

        for t in range(T):
            ng = len(gates)
            if t > 0:
                for g in range(ng):
                    for k in range(NKH):
                        nc.tensor.matmul(
                            gates[g][:],
                            hT_t[:, k * 128:(k + 1) * 128],
                            whc(k, g),
                            start=False, stop=(k == NKH - 1),
                        )
            # gate order: i, j, f, o
            it = apool.tile([128, H], f32, tag="ig")
            jt = apool.tile([128, H], f32, tag="jg")
            ft = apool.tile([128, H], f32, tag="fg")
            nc.scalar.activation(it[:], gates[0][:], AF.Sigmoid,
                                 bias=ait[:, t:t + 1])
            nc.scalar.activation(jt[:], gates[1][:], AF.Tanh)
            nc.scalar.activation(ft[:], gates[2][:], AF.Sigmoid,
                                 bias=aft[:, t:t + 1])
            p1 = tpool.tile([128, H], f32, tag="p1")
            nc.vector.tensor_mul(p1[:], it[:], jt[:])
            c_new = spool.tile([128, H], f32, tag="c")
            if t == 0:
                nc.vector.tensor_copy(c_new[:], p1[:])
            else:
                p2 = tpool.tile([128, H], f32, tag="p2")
                nc.vector.tensor_mul(p2[:], c_t[:], ft[:])
                nc.vector.tensor_add(c_new[:], p1[:], p2[:])
            c_t = c_new

            if t < T - 1:
                ot = apool.tile([128, H], f32, tag="og")
                nc.scalar.activation(ot[:], gates[3][:], AF.Sigmoid)
                tc_t = tpool.tile([128, H], f32, tag="tc")
                nc.scalar.activation(tc_t[:], c_t[:], AF.Tanh)
                hp = tpool.tile([128, H], f32r, tag="hp")
                nc.vector.tensor_mul(hp[:], tc_t[:], ot[:])
                # next step's x-projection fills PE while the h'
                # transposes wait on the ACT/DVE chain
                gates = emit_x(t + 1)
                pt = ppool.tile([128, H], f32r, tag="ht")
                for k in range(NKH):
                    nc.tensor.transpose(
                        pt[:, k * 128:(k + 1) * 128],
                        hp[:, k * 128:(k + 1) * 128], ident[:])
                hT_new = spool.tile([128, H], f32r, tag="hT")
                nc.vector.tensor_copy(hT_new[:], pt[:])
                hT_t = hT_new

        nc.sync.dma_start(cout[:], c_t[:])

        if not with_ff:
            lstm_es.close()
            nc.compile()
            return nc

        cc_in = dpool.tile([128, H], f32r)
        cc_all = dpool.tile([4, 128, H], f32r)
        nc.sync.dma_start(cc_in[:], c_t[:].bitcast(f32r))
        lstm_es.close()

        # ---------------- FF head ----------------
        nc.gpsimd.collective_compute(
            "AllGather", mybir.AluOpType.bypass,
            replica_groups=[[0, 1, 2, 3], [4, 5, 6, 7]],
            ins=[cc_in.opt()], outs=[cc_all.opt()],
        )
        with tc.tile_pool(name="ffw", bufs=1) as fpool, \
             tc.tile_pool(name="ffa", bufs=1) as fapool, \
             tc.tile_pool(name="ffp", bufs=4, space="PSUM") as fppool, \
             tc.tile_pool(name="ftp", bufs=2, space="PSUM") as ftppool:
            w1t = fpool.tile([128, 16 * FFD], f32r)
            for k in range(16):
                nc.sync.dma_start(w1t[:, k * FFD:(k + 1) * FFD], w1[k])
            w2t = fpool.tile([128, 8 * FFD], f32r)
            w3t = fpool.tile([128, 8 * FFD], f32r)
            for k in range(8):
                nc.sync.dma_start(w2t[:, k * FFD:(k + 1) * FFD], w2[k])
                nc.sync.dma_start(w3t[:, k * FFD:(k + 1) * FFD], w3[k])
            w4t = fpool.tile([128, 8 * 4], f32r)
            for k in range(8):
                nc.sync.dma_start(w4t[:, k * 4:(k + 1) * 4], w4[k])
            bfft = fpool.tile([1, 3 * FFD + 4], f32r)
            nc.sync.dma_start(bfft[:], bff[:])

            xcat = fapool.tile([128, 4 * H], f32r, tag="xcat")
            nc.sync.dma_start(xcat[:].rearrange("p (l j) -> p l j", l=4),
                              cc_all[:].rearrange("l p j -> p l j"))

            def transpose_to(src, nchunk, tag):
                """src [128, nchunk*128] f32r -> src^T chunk-concat."""
                dst = fapool.tile([128, nchunk * 128], f32r, tag=tag)
                for q in range(0, nchunk, 4):
                    qn = min(4, nchunk - q)
                    pt = ftppool.tile([128, 512], f32r, tag="ftp")
                    for k in range(qn):
                        nc.tensor.transpose(
                            pt[:, k * 128:(k + 1) * 128],
                            src[:, (q + k) * 128:(q + k + 1) * 128],
                            ident[:])
                    nc.vector.tensor_copy(
                        dst[:, q * 128:(q + qn) * 128], pt[:, :qn * 128])
                return dst

            def ff_layer(actT, nk, wt, wn, boff, bw, func, tag):
                """out = func(actT^T.T @ W + b); actT [128, nk*128]."""
                odt = f32 if func is None else f32r
                outs = fapool.tile([128, bw], odt, tag=tag)
                for n in range((bw + 511) // 512):
                    nn = min(512, bw - n * 512)
                    pg = fppool.tile([128, 512], f32, tag="ffg")
                    for k in range(nk):
                        nc.tensor.matmul(
                            pg[:, :nn],
                            actT[:, k * 128:(k + 1) * 128],
                            wt[:, k * wn + n * 512:k * wn + n * 512 + nn],
                            start=(k == 0), stop=False)
                    nc.tensor.matmul(
                        pg[:, :nn], ones1[:],
                        bfft[:, boff + n * 512:boff + n * 512 + nn],
                        start=False, stop=True)
                    if func is None:
                        nc.vector.tensor_copy(
                            outs[:, n * 512:n * 512 + nn], pg[:, :nn])
                    else:
                        nc.scalar.activation(
                            outs[:, n * 512:n * 512 + nn], pg[:, :nn], func)
                return outs

            xcatT = transpose_to(xcat, 16, "xcatT")
            h1 = ff_layer(xcatT, 16, w1t, FFD, 0, FFD, AF.Tanh, "h1")
            h1T = transpose_to(h1, 8, "h1T")
            h2 = ff_layer(h1T, 8, w2t, FFD, FFD, FFD, AF.Tanh, "h2")
            h2T = transpose_to(h2, 8, "h2T")
            h3 = ff_layer(h2T, 8, w3t, FFD, 2 * FFD, FFD, AF.Tanh, "h3")
            h3T = transpose_to(h3, 8, "h3T")
            lg = ff_layer(h3T, 8, w4t, 4, 3 * FFD, 4, None, "lg")
            nc.sync.dma_start(logits[:], lg[:])

    nc.compile()
    return nc


def pack_core_inputs(x_half, len_half, Wx, Wh, b, reverse,
                     W1, b1, W2, b2, W3, b3, W4, b4):
    """Build the in_map for one core. x_half [128, T, D] float32."""
    Tn = T
    if reverse:
        x_half = x_half[:, ::-1, :]
    pad = np.zeros((128, Tn, NKX * 128), np.float32)
    pad[:, :, :D] = x_half
    pad[:, :, D] = 1.0
    xT_ = np.ascontiguousarray(pad.transpose(1, 2, 0)).reshape(Tn, NKX, 128, 128)

    wxa = np.zeros((NKX * 128, G4), np.float32)
    wxa[:D] = Wx
    wxa[D] = b
    wx_ = np.ascontiguousarray(wxa.reshape(NKX, 128, G4))
    wh_ = np.ascontiguousarray(Wh.reshape(NKH, 128, G4))

    ts = np.arange(Tn)[None, :]
    tt = (Tn - 1 - ts) if reverse else ts
    m = tt < len_half[:, None]          # [128, T]
    addi_ = np.where(m, 0.0, -BIG).astype(np.float32)
    addf_ = (FORGET_BIAS + np.where(m, 0.0, BIG)).astype(np.float32)

    w1_ = np.ascontiguousarray(W1.reshape(16, 128, FFD))
    w2_ = np.ascontiguousarray(W2.reshape(8, 128, FFD))
    w3_ = np.ascontiguousarray(W3.reshape(8, 128, FFD))
    w4p = np.zeros((8, 128, 4), np.float32)
    w4p[:, :, :3] = W4.reshape(8, 128, 3)
    bff_ = np.zeros((1, 3 * FFD + 4), np.float32)
    bff_[0, :FFD] = b1
    bff_[0, FFD:2 * FFD] = b2
    bff_[0, 2 * FFD:3 * FFD] = b3
    bff_[0, 3 * FFD:3 * FFD + 3] = b4

    return {
        "xT": xT_, "wx": wx_, "wh": wh_,
        "addi": addi_, "addf": addf_,
        "identd": np.eye(128, dtype=np.float32),
        "onesd": np.ones((1, 128), np.float32),
        "w1": w1_, "w2": w2_, "w3": w3_, "w4": w4p, "bff": bff_,
    }


def make_in_maps(premises, hypotheses, premise_len, hypothesis_len,
                 p_fw_Wx, p_fw_Wh, p_fw_b, p_bw_Wx, p_bw_Wh, p_bw_b,
                 h_fw_Wx, h_fw_Wh, h_fw_b, h_bw_Wx, h_bw_Wh, h_bw_b,
                 W1, b1, W2, b2, W3, b3, W4, b4):
    premises = np.asarray(premises)[:, :T, :]
    hypotheses = np.asarray(hypotheses)[:, :T, :]
    ff = (W1, b1, W2, b2, W3, b3, W4, b4)
    in_maps = []
    for half in range(2):
        rows = slice(half * NB, (half + 1) * NB)
        for x, ln, Wx_, Wh_, b_, rev in [
            (premises, premise_len, p_fw_Wx, p_fw_Wh, p_fw_b, False),
            (premises, premise_len, p_bw_Wx, p_bw_Wh, p_bw_b, True),
            (hypotheses, hypothesis_len, h_fw_Wx, h_fw_Wh, h_fw_b, False),
            (hypotheses, hypothesis_len, h_bw_Wx, h_bw_Wh, h_bw_b, True),
        ]:
            in_maps.append(pack_core_inputs(
                np.asarray(x[rows]), np.asarray(ln[rows]),
                np.asarray(Wx_), np.asarray(Wh_), np.asarray(b_), rev, *ff))
    return in_maps


_NC_CACHE = {}


def get_nc(with_ff=True):
    key = (T, with_ff)
    if key not in _NC_CACHE:
        _NC_CACHE[key] = build(with_ff=with_ff)
    return _NC_CACHE[key]


def kernel(**inputs):
    in_maps = make_in_maps(**inputs)
    nc = get_nc()
    res = bass_utils.run_bass_kernel_spmd(nc, in_maps, core_ids=list(range(8)))
    out = np.empty((B, 3), np.float32)
    out[0:NB] = res.results[0]["logits"][:, :3]
    out[NB:2 * NB] = res.results[4]["logits"][:, :3]
    kernel.last_results = res
    return out
